# revision 4
# baseline (speedup 1.0000x reference)
"""ExpertLinear (MoE routing) Trainium2 Bass kernel.

y[b,:] = sum_k ew[b,k] * (x[b,:] @ W[k].T) + (ew @ bias)[b,:]

Strategy: 8-way data-parallel over the batch B across the 8 NeuronCores.
Per core (B_loc = 1024):
  - host supplies layout-prepped shards: xT [IN, B_loc] (x transposed),
    WT [K, IN, OUT] (weights transposed), ewT [K, B_loc], and the
    per-partition routing scalars ewp [128, B_loc/128, K]
  - matmul operands are bf16 (1 col/cycle like fp32r, but ~30% cheaper
    stationary loads and half the HBM traffic; rel err ~2.4e-3 vs the
    2e-2 budget):
        psum[b_tile, :] = sum_i xT[i, b_tile] @ WT[k, i, :]      (per expert k)
        y_acc[b, :]    += ACT(psum * ew[:, k])                   (per-partition scale)
    The second matmul of each (oh0, oh1) PSUM-bank pair reuses the
    stationary operand (ldweights=False).
  - bias term (ewT.T @ bias) is added at the end.
"""

import numpy as np
import ml_dtypes

from concourse import bacc
import concourse.mybir as mybir
import concourse.tile as tile
from concourse.bass_utils import run_bass_kernel_spmd

N_CORES = 8
B, K, OUT, IN = 8192, 8, 1024, 1024
P = 128

MM_DT = mybir.dt.bfloat16
NP_DT = ml_dtypes.bfloat16


def build_nc(b_loc=B // N_CORES, k=K, out_dim=OUT, in_dim=IN, mm_dt=MM_DT, rep=1,
             with_bias=True):
    nbt = b_loc // P      # batch tiles per core
    ni = in_dim // P      # contraction subtiles
    oh_sz = 512           # PSUM bank = 512 fp32
    noh = out_dim // oh_sz

    nc = bacc.Bacc()
    xt_d = nc.dram_tensor("xt", [in_dim, b_loc], mm_dt, kind="ExternalInput")
    wt_d = nc.dram_tensor("wt", [k, in_dim, out_dim], mm_dt, kind="ExternalInput")
    ewp_d = nc.dram_tensor("ewp", [P, nbt, k], mybir.dt.float32, kind="ExternalInput")
    ewt_d = nc.dram_tensor("ewt", [k, b_loc], mm_dt, kind="ExternalInput")
    bias_d = nc.dram_tensor("bias", [k, out_dim], mm_dt, kind="ExternalInput")
    y_d = nc.dram_tensor("y", [b_loc, out_dim], mybir.dt.float32, kind="ExternalOutput")

    with tile.TileContext(nc) as tc:
        with (
            tc.tile_pool(name="consts", bufs=1) as consts,
            tc.tile_pool(name="xt", bufs=1) as xt_pool,
            tc.tile_pool(name="yacc", bufs=1) as yacc_pool,
            tc.tile_pool(name="wbuf", bufs=2) as w_pool,
            tc.tile_pool(name="tmp", bufs=4) as tmp_pool,
            tc.tile_pool(name="ps_mm", bufs=4, space="PSUM") as ps_mm_pool,
        ):
            ewp_sb = consts.tile([P, nbt, k], mybir.dt.float32)
            nc.sync.dma_start(ewp_sb[:], ewp_d[:])
            ewt_sb = consts.tile([k, b_loc], mm_dt)
            nc.sync.dma_start(ewt_sb[:], ewt_d[:])
            bias_sb = consts.tile([k, out_dim], mm_dt)
            nc.sync.dma_start(bias_sb[:], bias_d[:])

            # xT resident, one tile per batch-tile so the first matmuls only
            # wait for their own slice: [128 (i_inner), ni (i_outer), P (b)]
            def load_xt(bt):
                xTbt = xt_pool.tile([P, ni, P], mm_dt, name=f"xT{bt}", tag=f"xT{bt}")
                nc.sync.dma_start(
                    xTbt[:],
                    xt_d[:, bt * P:(bt + 1) * P].rearrange("(io p) b -> p io b", p=P),
                )
                return xTbt

            def load_wchunks(kk):
                # W streamed in per-i chunks so matmuls start as soon as the
                # first contraction slice lands
                wchunks = []
                for i in range(ni):
                    wc = w_pool.tile(
                        [P, out_dim], mm_dt, name=f"wc{i}", tag=f"wc{i}"
                    )
                    nc.sync.dma_start(wc[:], wt_d[kk, i * P:(i + 1) * P, :])
                    wchunks.append(wc)
                return wchunks

            # DMA issue order shapes the critical path: xT[0] and expert-0's
            # W chunks go first so the first matmul series starts as early as
            # possible; the remaining batch tiles follow behind.
            xTs = [None] * nbt
            xTs[0] = load_xt(0)
            wchunks_k0 = load_wchunks(0)
            for bt in range(1, nbt):
                xTs[bt] = load_xt(bt)

            y_acc = yacc_pool.tile([P, nbt, out_dim], mybir.dt.float32)

            for _rep in range(rep):
                # Bias seed: y_acc = ewT.T @ bias. Skipped when the caller
                # knows bias == 0 (expert 0 then writes y_acc directly).
                if with_bias:
                    for bt in range(nbt):
                        pbias = ps_mm_pool.tile(
                            [P, noh, oh_sz], mybir.dt.float32,
                            name="pbias", tag="ps_mm",
                        )
                        for oh in range(noh):
                            nc.tensor.matmul(
                                pbias[:, oh, :],
                                ewt_sb[:, bt * P:(bt + 1) * P],
                                bias_sb[:, oh * oh_sz:(oh + 1) * oh_sz],
                                start=True,
                                stop=True,
                            )
                        for oh in range(noh):
                            nc.scalar.copy(
                                y_acc[:, bt, oh * oh_sz:(oh + 1) * oh_sz],
                                pbias[:, oh, :],
                            )

                # Main loop: stream each expert's WT once; accumulate over
                # the contraction (i) in PSUM, blend over experts (k) into
                # y_acc via ACT per-partition scale + DVE add.
                for kk in range(k):
                    if kk == 0 and _rep == 0:
                        wchunks = wchunks_k0
                    else:
                        wchunks = load_wchunks(kk)
                    for bt in range(nbt):
                        # one PSUM tile spanning both oh banks: the pair's
                        # matmuls share slot state, so the ldweights=False
                        # matmul is always scheduled directly after its
                        # weight-loading partner on the PE queue
                        pss = ps_mm_pool.tile(
                            [P, noh, oh_sz], mybir.dt.float32,
                            name="psmm", tag="ps_mm",
                        )
                        for i in range(ni):
                            lhsT = xTs[bt][:, i, :]
                            for oh in range(noh):
                                nc.tensor.matmul(
                                    pss[:, oh, :],
                                    lhsT,
                                    wchunks[i][:, oh * oh_sz:(oh + 1) * oh_sz],
                                    start=(i == 0),
                                    stop=(i == ni - 1),
                                )
                        for oh in range(noh):
                            osl = y_acc[:, bt, oh * oh_sz:(oh + 1) * oh_sz]
                            scale = ewp_sb[:, bt, kk:kk + 1]
                            if not with_bias and kk == 0:
                                # no bias seed: expert 0 writes y_acc directly
                                nc.scalar.mul(osl, pss[:, oh, :], scale)
                            else:
                                tmp = tmp_pool.tile([P, oh_sz], mybir.dt.float32)
                                nc.scalar.mul(tmp[:], pss[:, oh, :], scale)
                                nc.vector.tensor_add(osl, osl, tmp[:])
                        if kk == k - 1:
                            # y[bt] complete — stream it out while the
                            # remaining batch tiles finish
                            nc.sync.dma_start(
                                y_d[bt * P:(bt + 1) * P, :], y_acc[:, bt, :]
                            )

    nc.compile()
    # NOTE: no ldweights-reuse post-pass. Measured on HW, self-loading bf16
    # matmuls back-to-back run FASTER (224.6 ns/MM) than pairs patched with
    # ldweights=False (246.8 ns/MM) — the loads hide behind the previous
    # matmul's moving stream.
    return nc


_NC_CACHE = {}


def _get_nc(with_bias=True):
    key = ("bf16", with_bias)
    if key not in _NC_CACHE:
        _NC_CACHE[key] = build_nc(with_bias=with_bias)
    return _NC_CACHE[key]


def make_in_maps(x, ew, weight, bias):
    b_loc = B // N_CORES
    nbt = b_loc // P
    wt = np.ascontiguousarray(weight.transpose(0, 2, 1)).astype(NP_DT)  # [K, IN, OUT]
    bias16 = bias.astype(NP_DT)
    in_maps = []
    for c in range(N_CORES):
        xs = x[c * b_loc:(c + 1) * b_loc]
        xt = np.ascontiguousarray(xs.T).astype(NP_DT)  # [IN, b_loc]
        ews = ew[c * b_loc:(c + 1) * b_loc]  # [b_loc, K]
        ewp = np.ascontiguousarray(
            ews.reshape(nbt, P, K).transpose(1, 0, 2)
        )  # [P, nbt, K]
        ewt = np.ascontiguousarray(ews.T).astype(NP_DT)  # [K, b_loc]
        in_maps.append({"xt": xt, "wt": wt, "ewp": ewp, "ewt": ewt, "bias": bias16})
    return in_maps


def kernel(x, expert_weights, weight, bias):
    x = np.asarray(x, dtype=np.float32)
    ew = np.asarray(expert_weights, dtype=np.float32)
    weight = np.asarray(weight, dtype=np.float32)
    bias = np.asarray(bias, dtype=np.float32)

    nc = _get_nc(with_bias=bool(np.any(bias)))
    in_maps = make_in_maps(x, ew, weight, bias)
    last_exc = None
    for _attempt in range(3):
        try:
            res = run_bass_kernel_spmd(nc, in_maps, core_ids=list(range(N_CORES)))
            break
        except Exception as exc:  # transient device errors: retry
            last_exc = exc
    else:
        raise last_exc
    y = np.concatenate([r["y"] for r in res.results], axis=0)
    return y


# revision 8
# speedup vs baseline: 1.0035x; 1.0035x over previous
"""ExpertLinear (MoE routing) Trainium2 Bass kernel.

y[b,:] = sum_k ew[b,k] * (x[b,:] @ W[k].T) + (ew @ bias)[b,:]

Strategy: 8-way data-parallel over the batch B across the 8 NeuronCores.
Per core (B_loc = 1024):
  - host supplies layout-prepped shards: xT [IN, B_loc] (x transposed),
    WT [K, IN, OUT] (weights transposed), ewT [K, B_loc], and the
    per-partition routing scalars ewp [128, B_loc/128, K]
  - matmul operands are bf16 (1 col/cycle like fp32r, half the HBM traffic
    and SBUF footprint; rel err ~2.4e-3 vs the 2e-2 budget):
        psum[b_tile, :] = sum_i xT[i, b_tile] @ WT[k, i, :]      (per expert k)
        y_acc[b, :]    += ACT(psum * ew[:, k])                   (per-partition scale)
  - ALL eight experts' weights are SBUF-resident (8 x 16KB/partition in
    bf16): measured on HW, concurrent W streaming slows the matmul pipe
    from ~189 ns/MM to ~263 ns/MM, so weights are loaded once up front
    (overlapped with expert-0 compute) and reused thereafter.
  - bias term (ewT.T @ bias) is added at the end.
"""

import numpy as np
import ml_dtypes

from concourse import bacc
import concourse.mybir as mybir
import concourse.tile as tile
from concourse.bass_utils import run_bass_kernel_spmd

N_CORES = 8
B, K, OUT, IN = 8192, 8, 1024, 1024
P = 128

MM_DT = mybir.dt.bfloat16
NP_DT = ml_dtypes.bfloat16


def build_nc(b_loc=B // N_CORES, k=K, out_dim=OUT, in_dim=IN, mm_dt=MM_DT, rep=1,
             with_bias=True):
    nbt = b_loc // P      # batch tiles per core
    ni = in_dim // P      # contraction subtiles
    oh_sz = 512           # PSUM bank = 512 fp32
    noh = out_dim // oh_sz

    nc = bacc.Bacc()
    xt_d = nc.dram_tensor("xt", [in_dim, b_loc], mm_dt, kind="ExternalInput")
    wt_d = nc.dram_tensor("wt", [k, in_dim, out_dim], mm_dt, kind="ExternalInput")
    ewp_d = nc.dram_tensor("ewp", [P, nbt, k], mybir.dt.float32, kind="ExternalInput")
    ewt_d = nc.dram_tensor("ewt", [k, b_loc], mm_dt, kind="ExternalInput")
    bias_d = nc.dram_tensor("bias", [k, out_dim], mm_dt, kind="ExternalInput")
    y_d = nc.dram_tensor("y", [b_loc, out_dim], mybir.dt.float32, kind="ExternalOutput")

    with tile.TileContext(nc) as tc:
        with (
            tc.tile_pool(name="consts", bufs=1) as consts,
            tc.tile_pool(name="xt", bufs=1) as xt_pool,
            tc.tile_pool(name="yacc", bufs=1) as yacc_pool,
            tc.tile_pool(name="wbuf", bufs=1) as w_pool,
            tc.tile_pool(name="tmp", bufs=4) as tmp_pool,
            tc.tile_pool(name="ps_mm", bufs=4, space="PSUM") as ps_mm_pool,
        ):
            ewp_sb = consts.tile([P, nbt, k], mybir.dt.float32)
            nc.sync.dma_start(ewp_sb[:], ewp_d[:])
            ewt_sb = consts.tile([k, b_loc], mm_dt)
            nc.sync.dma_start(ewt_sb[:], ewt_d[:])
            bias_sb = consts.tile([k, out_dim], mm_dt)
            nc.sync.dma_start(bias_sb[:], bias_d[:])

            # xT resident, one tile per batch-tile so the first matmuls only
            # wait for their own slice: [128 (i_inner), ni (i_outer), P (b)]
            def load_xt(bt):
                xTbt = xt_pool.tile([P, ni, P], mm_dt, name=f"xT{bt}", tag=f"xT{bt}")
                nc.sync.dma_start(
                    xTbt[:],
                    xt_d[:, bt * P:(bt + 1) * P].rearrange("(io p) b -> p io b", p=P),
                )
                return xTbt

            def load_wchunks(kk):
                # W loaded in per-i chunks so matmuls start as soon as the
                # first contraction slice lands; tiles are persistent (one
                # set per expert), so reps > 0 run with zero W DMA traffic.
                wchunks = []
                for i in range(ni):
                    wc = w_pool.tile(
                        [P, out_dim], mm_dt, name=f"wc{kk}_{i}", tag=f"wc{kk}_{i}"
                    )
                    nc.sync.dma_start(wc[:], wt_d[kk, i * P:(i + 1) * P, :])
                    wchunks.append(wc)
                return wchunks

            # DMA issue order shapes the critical path: xT[0] and expert-0's
            # W chunks go first so the first matmul series starts as early as
            # possible; the remaining batch tiles and experts follow behind.
            xTs = [None] * nbt
            xTs[0] = load_xt(0)
            wchunks_all = [None] * k
            wchunks_all[0] = load_wchunks(0)
            for bt in range(1, nbt):
                xTs[bt] = load_xt(bt)

            y_acc = yacc_pool.tile([P, nbt, out_dim], mybir.dt.float32)

            for _rep in range(rep):
                # Bias seed: y_acc = ewT.T @ bias. Skipped when the caller
                # knows bias == 0 (expert 0 then writes y_acc directly).
                if with_bias:
                    for bt in range(nbt):
                        pbias = ps_mm_pool.tile(
                            [P, noh, oh_sz], mybir.dt.float32,
                            name="pbias", tag="ps_mm",
                        )
                        for oh in range(noh):
                            nc.tensor.matmul(
                                pbias[:, oh, :],
                                ewt_sb[:, bt * P:(bt + 1) * P],
                                bias_sb[:, oh * oh_sz:(oh + 1) * oh_sz],
                                start=True,
                                stop=True,
                            )
                        for oh in range(noh):
                            nc.scalar.copy(
                                y_acc[:, bt, oh * oh_sz:(oh + 1) * oh_sz],
                                pbias[:, oh, :],
                            )

                # Main loop: stream each expert's WT once; accumulate over
                # the contraction (i) in PSUM, blend over experts (k) into
                # y_acc via ACT per-partition scale + DVE add.
                for kk in range(k):
                    if wchunks_all[kk] is None:
                        wchunks_all[kk] = load_wchunks(kk)
                    wchunks = wchunks_all[kk]
                    for bt in range(nbt):
                        # one PSUM tile spanning both oh banks: the pair's
                        # matmuls share slot state, so the ldweights=False
                        # matmul is always scheduled directly after its
                        # weight-loading partner on the PE queue
                        pss = ps_mm_pool.tile(
                            [P, noh, oh_sz], mybir.dt.float32,
                            name="psmm", tag="ps_mm",
                        )
                        for i in range(ni):
                            lhsT = xTs[bt][:, i, :]
                            for oh in range(noh):
                                nc.tensor.matmul(
                                    pss[:, oh, :],
                                    lhsT,
                                    wchunks[i][:, oh * oh_sz:(oh + 1) * oh_sz],
                                    start=(i == 0),
                                    stop=(i == ni - 1),
                                )
                        for oh in range(noh):
                            osl = y_acc[:, bt, oh * oh_sz:(oh + 1) * oh_sz]
                            scale = ewp_sb[:, bt, kk:kk + 1]
                            if not with_bias and kk == 0:
                                # no bias seed: expert 0 writes y_acc directly
                                nc.scalar.mul(osl, pss[:, oh, :], scale)
                            else:
                                tmp = tmp_pool.tile([P, oh_sz], mybir.dt.float32)
                                nc.scalar.mul(tmp[:], pss[:, oh, :], scale)
                                nc.vector.tensor_add(osl, osl, tmp[:])
                        if kk == k - 1:
                            # y[bt] complete — stream it out while the
                            # remaining batch tiles finish
                            nc.sync.dma_start(
                                y_d[bt * P:(bt + 1) * P, :], y_acc[:, bt, :]
                            )

    nc.compile()
    # NOTE: no ldweights-reuse post-pass. Measured on HW, self-loading bf16
    # matmuls back-to-back run FASTER (224.6 ns/MM) than pairs patched with
    # ldweights=False (246.8 ns/MM) — the loads hide behind the previous
    # matmul's moving stream.
    return nc


_NC_CACHE = {}


def _get_nc(with_bias=True):
    key = ("bf16", with_bias)
    if key not in _NC_CACHE:
        _NC_CACHE[key] = build_nc(with_bias=with_bias)
    return _NC_CACHE[key]


def make_in_maps(x, ew, weight, bias):
    b_loc = B // N_CORES
    nbt = b_loc // P
    wt = np.ascontiguousarray(weight.transpose(0, 2, 1)).astype(NP_DT)  # [K, IN, OUT]
    bias16 = bias.astype(NP_DT)
    in_maps = []
    for c in range(N_CORES):
        xs = x[c * b_loc:(c + 1) * b_loc]
        xt = np.ascontiguousarray(xs.T).astype(NP_DT)  # [IN, b_loc]
        ews = ew[c * b_loc:(c + 1) * b_loc]  # [b_loc, K]
        ewp = np.ascontiguousarray(
            ews.reshape(nbt, P, K).transpose(1, 0, 2)
        )  # [P, nbt, K]
        ewt = np.ascontiguousarray(ews.T).astype(NP_DT)  # [K, b_loc]
        in_maps.append({"xt": xt, "wt": wt, "ewp": ewp, "ewt": ewt, "bias": bias16})
    return in_maps


def kernel(x, expert_weights, weight, bias):
    x = np.asarray(x, dtype=np.float32)
    ew = np.asarray(expert_weights, dtype=np.float32)
    weight = np.asarray(weight, dtype=np.float32)
    bias = np.asarray(bias, dtype=np.float32)

    nc = _get_nc(with_bias=bool(np.any(bias)))
    in_maps = make_in_maps(x, ew, weight, bias)
    last_exc = None
    for _attempt in range(3):
        try:
            res = run_bass_kernel_spmd(nc, in_maps, core_ids=list(range(N_CORES)))
            break
        except Exception as exc:  # transient device errors: retry
            last_exc = exc
    else:
        raise last_exc
    y = np.concatenate([r["y"] for r in res.results], axis=0)
    return y
